# revision 1
# baseline (speedup 1.0000x reference)
"""Trainium2 Bass kernel for a teacher-forced GRU decoder + log_softmax.

Model (PyTorch GRU cell semantics, gates ordered r,z,n):
    x = emb[target[:, :-1]]; h0 = encoder_hidden[0]
    scan over T-1 steps -> hs; logp = log_softmax(hs @ out_W.T + out_b)

Strategy over 8 NeuronCores:
  * GRU recurrence is replicated on every core with the full batch (its cost
    is W_hh weight-streaming-bound, so batch sharding would not help and
    replication avoids any hidden-state communication).
  * The projection/log_softmax (the memory-bound part: 520MB of output) is
    vocab-sharded: core j computes logits/logp for vocab [4000j, 4000j+4000).
    The softmax denominator is completed with one small (few-KB) AllReduce
    per group of 8 position-tiles.
  * All heavy matmuls run in bf16 (weights pre-transposed/cast on host);
    gate math, softmax sums and the final output are fp32.
    ln(S) is evaluated as 15*ln2 + ln1p(S/32768 - 1) via a short Taylor
    series so the whole kernel uses a single ACT table set (exp/tanh).
"""
import sys
sys.path.insert(0, "/opt/trn_rl_repo")
import numpy as np
import ml_dtypes

import concourse.bass as bass
import concourse.bacc as bacc
import concourse.mybir as mybir
from concourse import tile
from concourse.bass_utils import run_bass_kernel_spmd

BF16 = ml_dtypes.bfloat16
F32 = np.float32
N_CORES = 8
HID = 512
EMB = 512
BATCH = 32
VOCAB = 32000
VSHARD = VOCAB // N_CORES      # 4000
VBANK = 500                    # psum bank width (f32)
NBANK = VSHARD // VBANK        # 8
KC = HID // 128                # 4 k-chunks
MC = 3 * HID // 128            # 12 m-chunks
LN2 = float(np.log(2.0))


def build_nc(T):
    POS = BATCH * T
    P_PAD = ((POS + 127) // 128) * 128
    TILE_P = 127 if POS % 127 == 0 else 128
    assert POS % TILE_P == 0
    NT = POS // TILE_P
    GROUPS = [list(range(g, min(g + 8, NT))) for g in range(0, NT, 8)]
    NTG = P_PAD // 128

    nc = bacc.Bacc("TRN2", target_bir_lowering=False, debug=False,
                   num_devices=N_CORES)
    dt = mybir.dt
    def param(name, shape, d, out=False):
        return nc.declare_dram_parameter(name, list(shape), d, isOutput=out)

    idx = param("idx", [128, NTG], dt.int32)
    ident = param("ident", [128, 128], dt.bfloat16)
    emb = param("emb", [VOCAB, EMB], dt.bfloat16)
    wih = param("wih", [128, KC, 3 * HID], dt.bfloat16)
    whh = param("whh", [128, KC, 3 * HID], dt.bfloat16)
    bgi = param("bgi", [128, MC], dt.float32)
    bhn = param("bhn", [128, KC, BATCH], dt.float32)
    h0 = param("h0", [128, KC, BATCH], dt.bfloat16)
    wout = param("wout", [128, KC, VSHARD], dt.bfloat16)
    outb = param("outb", [1, VSHARD], dt.float32)
    out = param("out", [POS, VSHARD], dt.float32, out=True)

    AF = mybir.ActivationFunctionType
    AL = mybir.AluOpType
    ts = bass.ts

    # split phase-1 position range into halves to bound xt SBUF usage
    SPLIT = ((POS // 2 + 127) // 128) * 128
    HALves = [(0, POS)] if POS <= 2048 else [(0, SPLIT), (SPLIT, POS)]
    XT_COLS = max(((hi - lo + 127) // 128) * 128 for lo, hi in HALves)

    with tile.TileContext(nc) as tc:
        with tc.tile_pool(name="persist", bufs=1) as pp:
            hsT = pp.tile([128, KC, POS], dt.bfloat16)
            bgi_sb = pp.tile([128, MC], dt.float32)
            bhn_sb = pp.tile([128, KC, BATCH], dt.float32)
            h0_sb = pp.tile([128, KC, BATCH], dt.bfloat16)
            half_sb = pp.tile([128, 1], dt.float32)
            nc.vector.memset(half_sb[:], 0.5)
            nc.sync.dma_start(bgi_sb[:], bgi[:])
            nc.sync.dma_start(bhn_sb[:], bhn[:])
            nc.sync.dma_start(h0_sb[:], h0[:])

          # ---- gi zone: GI + recurrence, freed before projection ----
            with tc.tile_pool(name="gizone", bufs=1) as gz:
                gi_sb = gz.tile([128, MC, POS], dt.bfloat16)
                whh_sb = gz.tile([128, KC, 3 * HID], dt.bfloat16)
                nc.sync.dma_start(whh_sb[:], whh[:])

                # ---------- Phase 1: embedding gather + GI ----------
                with tc.tile_pool(name="giph", bufs=1) as gp, \
                     tc.tile_pool(name="xg", bufs=3) as xg, \
                     tc.tile_pool(name="gipsum", bufs=4, space="PSUM") as gps:
                    xt = gp.tile([128, KC, XT_COLS], dt.bfloat16)
                    wih_sb = gp.tile([128, KC, 3 * HID], dt.bfloat16)
                    idx_sb = gp.tile([128, NTG], dt.int32)
                    ident_sb = gp.tile([128, 128], dt.bfloat16)
                    nc.sync.dma_start(idx_sb[:], idx[:])
                    nc.sync.dma_start(ident_sb[:], ident[:])
                    nc.sync.dma_start(wih_sb[:], wih[:])
                    for lo, hi in HALves:
                        i0 = lo // 128
                        for i in range(i0, (hi + 127) // 128):
                            xrow = xg.tile([128, EMB], dt.bfloat16, tag="xrow")
                            nc.gpsimd.indirect_dma_start(
                                out=xrow[:], out_offset=None, in_=emb[:],
                                in_offset=bass.IndirectOffsetOnAxis(
                                    ap=idx_sb[:, i:i + 1], axis=0))
                            for kc in range(KC):
                                tp = gps.tile([128, 128], dt.bfloat16, tag="tp")
                                nc.tensor.transpose(
                                    tp[:], xrow[:, kc * 128:(kc + 1) * 128],
                                    ident_sb[:])
                                nc.scalar.copy(
                                    xt[:, kc, ts(i - i0, 128)], tp[:])
                        for mc in range(MC):
                            for p0 in range(lo, hi, 508):
                                blk = min(508, hi - p0)
                                ps = gps.tile([128, 508], dt.float32, tag="gps")
                                for kc in range(KC):
                                    nc.tensor.matmul(
                                        ps[:, 0:blk],
                                        wih_sb[:, kc, mc * 128:(mc + 1) * 128],
                                        xt[:, kc, p0 - lo:p0 - lo + blk],
                                        start=(kc == 0), stop=(kc == KC - 1))
                                nc.scalar.activation(
                                    gi_sb[:, mc, p0:p0 + blk], ps[:, 0:blk],
                                    AF.Identity, bias=bgi_sb[:, mc:mc + 1])

                # ------- Phase 2: GRU recurrence (replicated, full batch) --
                with tc.tile_pool(name="rec", bufs=2) as rp, \
                     tc.tile_pool(name="hbuf", bufs=2) as hp, \
                     tc.tile_pool(name="recpsum", bufs=2, space="PSUM") as rps:
                    h_cur = None
                    for t in range(T):
                        ps = rps.tile([128, MC, BATCH], dt.float32, tag="gh")
                        for mc in range(MC):
                            for kc in range(KC):
                                rhs = (h0_sb[:, kc, :] if t == 0
                                       else hsT[:, kc, ts(t - 1, BATCH)])
                                nc.tensor.matmul(
                                    ps[:, mc, :],
                                    whh_sb[:, kc, mc * 128:(mc + 1) * 128],
                                    rhs, start=(kc == 0), stop=(kc == KC - 1))
                        u_rz = rp.tile([128, 8, BATCH], dt.float32, tag="urz")
                        nc.vector.tensor_tensor(
                            u_rz[:], ps[:, 0:8, :], gi_sb[:, 0:8, ts(t, BATCH)], AL.add)
                        t_rz = rp.tile([128, 8, BATCH], dt.float32, tag="trz")
                        nc.scalar.activation(t_rz[:], u_rz[:], AF.Tanh, scale=0.5)
                        rz = rp.tile([128, 8, BATCH], dt.float32, tag="rz")
                        nc.scalar.activation(rz[:], t_rz[:], AF.Identity,
                                             scale=0.5, bias=half_sb[:])
                        u_n = rp.tile([128, KC, BATCH], dt.float32, tag="un")
                        nc.vector.tensor_tensor(u_n[:], ps[:, 8:12, :], bhn_sb[:], AL.add)
                        v = rp.tile([128, KC, BATCH], dt.float32, tag="v")
                        nc.vector.tensor_tensor(v[:], u_n[:], rz[:, 0:4, :], AL.mult)
                        t2 = rp.tile([128, KC, BATCH], dt.float32, tag="t2")
                        nc.vector.tensor_tensor(t2[:], v[:], gi_sb[:, 8:12, ts(t, BATCH)], AL.add)
                        n_g = rp.tile([128, KC, BATCH], dt.float32, tag="ng")
                        nc.scalar.activation(n_g[:], t2[:], AF.Tanh)
                        d = rp.tile([128, KC, BATCH], dt.float32, tag="d")
                        nc.vector.tensor_tensor(
                            d[:], h0_sb[:] if t == 0 else h_cur[:], n_g[:], AL.subtract)
                        zd = rp.tile([128, KC, BATCH], dt.float32, tag="zd")
                        nc.vector.tensor_tensor(zd[:], rz[:, 4:8, :], d[:], AL.mult)
                        h_new = hp.tile([128, KC, BATCH], dt.float32, tag="h")
                        nc.vector.tensor_tensor(h_new[:], n_g[:], zd[:], AL.add)
                        nc.vector.tensor_copy(hsT[:, :, ts(t, BATCH)], h_new[:])
                        h_cur = h_new

            # ---------- Phase 3: projection + log_softmax ----------------
            with tc.tile_pool(name="proj", bufs=1) as jp, \
                 tc.tile_pool(name="projs", bufs=2) as js, \
                 tc.tile_pool(name="escr", bufs=3) as ep, \
                 tc.tile_pool(name="ostage", bufs=4) as op, \
                 tc.tile_pool(name="projpsum", bufs=4, space="PSUM") as pps, \
                 tc.tile_pool(name="ardram", bufs=2, space="DRAM") as ad:
                wout_sb = jp.tile([128, KC, VSHARD], dt.bfloat16)
                outb_sb = jp.tile([1, VSHARD], dt.float32)
                ones_sb = jp.tile([1, TILE_P], dt.float32)
                nc.sync.dma_start(wout_sb[:], wout[:])
                nc.sync.dma_start(outb_sb[:], outb[:])
                nc.vector.memset(ones_sb[:], 1.0)
                Lbuf = jp.tile([128, 8, VSHARD], dt.bfloat16)

                for grp in GROUPS:
                    ng = len(grp)
                    sums = js.tile([128, 8 * NBANK], dt.float32, tag="sums")
                    for gi_i, p in enumerate(grp):
                        for vb in range(NBANK):
                            ps = pps.tile([TILE_P, VBANK], dt.float32, tag="pj")
                            nc.tensor.matmul(
                                ps[:], ones_sb[:, 0:TILE_P],
                                outb_sb[:, ts(vb, VBANK)],
                                start=True, stop=False)
                            for kc in range(KC):
                                nc.tensor.matmul(
                                    ps[:],
                                    hsT[:, kc, ts(p, TILE_P)],
                                    wout_sb[:, kc, ts(vb, VBANK)],
                                    start=False, stop=(kc == KC - 1))
                            nc.vector.tensor_copy(
                                Lbuf[0:TILE_P, gi_i, ts(vb, VBANK)], ps[:])
                            esc = ep.tile([TILE_P, VBANK], dt.bfloat16, tag="esc")
                            nc.scalar.activation(
                                esc[:], Lbuf[0:TILE_P, gi_i, ts(vb, VBANK)],
                                AF.Exp,
                                accum_out=sums[0:TILE_P, gi_i * NBANK + vb:
                                               gi_i * NBANK + vb + 1])
                    s8 = js.tile([128, 8], dt.float32, tag="s8")
                    for gi_i in range(ng):
                        nc.vector.tensor_reduce(
                            s8[0:TILE_P, gi_i:gi_i + 1],
                            sums[0:TILE_P, ts(gi_i, NBANK)],
                            mybir.AxisListType.X, AL.add)
                    arin = ad.tile([TILE_P, 8], dt.float32, tag="arin")
                    arout = ad.tile([TILE_P, 8], dt.float32, tag="arout",
                                    addr_space="Shared")
                    nc.gpsimd.dma_start(arin[:], s8[0:TILE_P, :])
                    nc.gpsimd.collective_compute(
                        "AllReduce", AL.add,
                        replica_groups=[list(range(N_CORES))],
                        ins=[arin.opt()], outs=[arout.opt()])
                    stot = js.tile([128, 8], dt.float32, tag="stot")
                    nc.gpsimd.dma_start(stot[0:TILE_P, :], arout[:])
                    # c = ln(stot) = 15*ln2 + ln1p(u), u = stot/32768 - 1
                    u = js.tile([128, 8], dt.float32, tag="u")
                    nc.vector.tensor_scalar(u[0:TILE_P, :], stot[0:TILE_P, :],
                                            1.0 / 32768.0, -1.0, AL.mult, AL.add)
                    acc = js.tile([128, 8], dt.float32, tag="acc")
                    nc.vector.tensor_scalar(acc[0:TILE_P, :], u[0:TILE_P, :],
                                            0.2, -0.25, AL.mult, AL.add)
                    for cst in (1.0 / 3.0, -0.5, 1.0):
                        t1 = js.tile([128, 8], dt.float32, tag="hrn")
                        nc.vector.tensor_tensor(t1[0:TILE_P, :], acc[0:TILE_P, :],
                                                u[0:TILE_P, :], AL.mult)
                        acc = js.tile([128, 8], dt.float32, tag="acc")
                        nc.vector.tensor_scalar(acc[0:TILE_P, :], t1[0:TILE_P, :],
                                                cst, None, AL.add)
                    cfin = js.tile([128, 8], dt.float32, tag="cfin")
                    nc.vector.tensor_tensor(cfin[0:TILE_P, :], acc[0:TILE_P, :],
                                            u[0:TILE_P, :], AL.mult)
                    c_ap = js.tile([128, 8], dt.float32, tag="cap")
                    nc.vector.tensor_scalar(c_ap[0:TILE_P, :], cfin[0:TILE_P, :],
                                            15.0 * LN2, None, AL.add)
                    for gi_i, p in enumerate(grp):
                        for vb in range(NBANK):
                            o = op.tile([TILE_P, VBANK], dt.float32, tag="o")
                            nc.vector.tensor_scalar(
                                o[:], Lbuf[0:TILE_P, gi_i, ts(vb, VBANK)],
                                c_ap[0:TILE_P, gi_i:gi_i + 1], None, AL.subtract)
                            nc.sync.dma_start(
                                out[ts(p, TILE_P), ts(vb, VBANK)], o[:])
    nc.compile()
    return nc


def prep_inputs(target, encoder_hidden, emb_weight, W_ih, W_hh, b_ih, b_hh,
                out_W, out_b):
    T = target.shape[1] - 1
    POS = BATCH * T
    P_PAD = ((POS + 127) // 128) * 128

    tok = np.ascontiguousarray(target[:, :T].T).reshape(-1).astype(np.int32)
    tok_pad = np.zeros(P_PAD, np.int32)
    tok_pad[:POS] = tok
    idx = np.ascontiguousarray(tok_pad.reshape(P_PAD // 128, 128).T)
    ident = np.eye(128, dtype=BF16)

    emb_bf = emb_weight.astype(BF16)

    def chunkT(w):  # [512, M] -> [128, 4, M]
        return np.ascontiguousarray(w.reshape(KC, 128, -1).transpose(1, 0, 2))

    wihT = chunkT(np.ascontiguousarray(W_ih.T.astype(BF16)))
    whhT = chunkT(np.ascontiguousarray(W_hh.T.astype(BF16)))

    bgi_vec = b_ih.astype(np.float64) + np.concatenate(
        [b_hh[:2 * HID], np.zeros(HID)]).astype(np.float64)
    bgi = np.ascontiguousarray(bgi_vec.astype(F32).reshape(MC, 128).T)
    bhn = np.ascontiguousarray(np.broadcast_to(
        b_hh[2 * HID:].astype(F32).reshape(KC, 128).transpose(1, 0)[:, :, None],
        (128, KC, BATCH)))
    h0 = chunkT(np.ascontiguousarray(encoder_hidden[0].T.astype(BF16)))

    outWT = np.ascontiguousarray(out_W.T.astype(BF16))

    in_maps = []
    for j in range(N_CORES):
        sl = slice(j * VSHARD, (j + 1) * VSHARD)
        in_maps.append({
            "idx": idx, "ident": ident, "emb": emb_bf, "wih": wihT,
            "whh": whhT, "bgi": bgi, "bhn": bhn, "h0": h0,
            "wout": chunkT(outWT[:, sl]),
            "outb": out_b[sl].astype(F32).reshape(1, -1),
        })
    return in_maps


_NC_CACHE = {}


def kernel(**inputs):
    inputs = {k: np.asarray(v) for k, v in inputs.items()}
    target = inputs["target"].astype(np.int32)
    T = target.shape[1] - 1
    if T not in _NC_CACHE:
        _NC_CACHE[T] = build_nc(T)
    nc = _NC_CACHE[T]
    in_maps = prep_inputs(
        target, inputs["encoder_hidden"].astype(F32),
        inputs["emb_weight"].astype(F32), inputs["W_ih"].astype(F32),
        inputs["W_hh"].astype(F32), inputs["b_ih"].astype(F32),
        inputs["b_hh"].astype(F32), inputs["out_W"].astype(F32),
        inputs["out_b"].astype(F32))
    res = run_bass_kernel_spmd(nc, in_maps, list(range(N_CORES)))
    full = np.concatenate(
        [res.results[j]["out"] for j in range(N_CORES)], axis=1)
    return np.ascontiguousarray(full.reshape(T, BATCH, VOCAB))



# revision 7
# speedup vs baseline: 3.8061x; 3.8061x over previous
"""Trainium2 Bass kernel for a teacher-forced GRU decoder + log_softmax.

Model (PyTorch GRU cell semantics, gates ordered r,z,n):
    x = emb[target[:, :-1]]; h0 = encoder_hidden[0]
    scan over T-1=127 steps -> hs; logp = log_softmax(hs @ out_W.T + out_b)

v2 strategy over 8 NeuronCores (SPMD, one program, per-core data differs
only in the vocab shard of out_W/out_b):
  * Host precomputes embgi = emb @ W_ih.T (+ input biases folded; the n-gate
    part doubled so the 0.5 activation-scale trick works).  On device the
    per-step gate inputs are gathered by token (indirect DMA, row-major) and
    injected into PSUM with tiny identity matmuls -- the PE does the
    transpose for free, no separate GI phase.
  * GRU recurrence: W_hh in fp8-e4m3 with DoubleRow matmuls; gate math uses
    2*sigmoid(x) = 1 + tanh(x/2) so a single ACT table set (exp/tanh)
    serves the whole kernel.  Batch is split in two halves whose serial
    chains interleave across engines to hide cross-engine semaphore
    latency.
  * Projection/log_softmax is vocab-sharded (4000 cols/core, padded to
    4096 with bias -30) and software-pipelined INTO the recurrence: every
    recurrence step emits 1/4 of the projection work for the newest
    complete 127-position tile (fp8 DoubleRow matmuls, exp+accumulate on
    the scalar engine, PSUM->SBUF drains split across DVE/Act, final
    subtract on the otherwise idle Pool engine, bf16 output).
  * The softmax denominator is finished with one tiny (127x3 f32)
    AllReduce per group of 3 position-tiles, consumed ~4 steps later so
    its latency stays off the critical path.
"""
import sys
sys.path.insert(0, "/opt/trn_rl_repo")
import numpy as np
import ml_dtypes

import concourse.bass as bass
import concourse.bacc as bacc
import concourse.mybir as mybir
from concourse import tile
from concourse.bass_utils import run_bass_kernel_spmd

BF16 = ml_dtypes.bfloat16
FP8 = ml_dtypes.float8_e4m3
F32 = np.float32
N_CORES = 8
HID = 512
EMB = 512
BATCH = 32
VOCAB = 32000
T = 127
POS = BATCH * T              # 4064
VSHARD = VOCAB // N_CORES    # 4000
VPAD = 4096                  # padded shard width (pad bias -30 -> exp ~ 0)
KC = HID // 128              # 4 k-chunks
TILE_P = 127                 # positions per projection tile
NT = POS // TILE_P           # 32 tiles
GT = 3                       # tiles per AllReduce group
NGRP = (NT + GT - 1) // GT   # 11 groups (last has 2 tiles)
LN2 = float(np.log(2.0))
DR = mybir.MatmulPerfMode.DoubleRow


def build_nc():
    nc = bacc.Bacc("TRN2", target_bir_lowering=False, debug=False,
                   num_devices=N_CORES)
    dt = mybir.dt
    AF = mybir.ActivationFunctionType
    AL = mybir.AluOpType
    ts = bass.ts

    def param(name, shape, d, out=False):
        return nc.declare_dram_parameter(name, list(shape), d, isOutput=out)

    idx = param("idx", [128, 32], dt.int32)
    ident = param("ident", [128, 128], dt.bfloat16)
    embgi = param("embgi", [VOCAB, 3 * HID], dt.bfloat16)
    whh8 = param("whh8", [128, KC, 3 * HID], dt.float8e4)
    bhn = param("bhn", [128, KC, 16], dt.bfloat16)
    h0 = param("h0", [128, KC, BATCH], dt.bfloat16)
    h08 = param("h08", [128, KC, BATCH], dt.float8e4)
    wout8 = param("wout8", [128, KC, VPAD], dt.float8e4)
    outb = param("outb", [1, VPAD], dt.bfloat16)
    out = param("out", [POS, VPAD], dt.bfloat16, out=True)

    # ---- schedule: per recurrence step, the projection/softmax actions ----
    # tile p chunk-pair q (1024 vocab cols) is emitted after step 3+4p+q.
    sched = {}

    def at(t, *action):
        sched.setdefault(t, []).append(action)

    for p in range(NT):
        for q in range(4):
            at(3 + 4 * p + q, "chunk", p, q)
    for g in range(NGRP):
        tiles = list(range(g * GT, min((g + 1) * GT, NT)))
        t_last = 3 + 4 * tiles[-1] + 3
        at(t_last, "allreduce", g, len(tiles))
        at(t_last + 4, "fetch_c", g, len(tiles))
        for k, p in enumerate(tiles):
            at(t_last + 5 + k, "sub", g, p)

    with tile.TileContext(nc) as tc:
        with tc.tile_pool(name="persist", bufs=1) as pp, \
             tc.tile_pool(name="xg", bufs=4) as xg, \
             tc.tile_pool(name="rscr", bufs=2) as rp, \
             tc.tile_pool(name="lbuf", bufs=2) as lb, \
             tc.tile_pool(name="esc", bufs=2) as ep, \
             tc.tile_pool(name="ostage", bufs=3) as op, \
             tc.tile_pool(name="smalls", bufs=2) as sp, \
             tc.tile_pool(name="recpsum", bufs=2, space="PSUM") as rps, \
             tc.tile_pool(name="pjpsum", bufs=2, space="PSUM") as pps, \
             tc.tile_pool(name="ardram", bufs=2, space="DRAM") as ad:

            hsT = pp.tile([128, KC, POS], dt.bfloat16)
            hsT8 = pp.tile([128, KC, POS], dt.float8e4)
            whh_sb = pp.tile([128, KC, 3 * HID], dt.float8e4)
            wout_sb = pp.tile([128, KC, VPAD], dt.float8e4)
            ident_sb = pp.tile([128, 128], dt.bfloat16)
            idx_sb = pp.tile([128, 32], dt.int32)
            bhn_sb = pp.tile([128, KC, 16], dt.bfloat16)
            h0_sb = pp.tile([128, KC, BATCH], dt.bfloat16)
            h08_sb = pp.tile([128, KC, BATCH], dt.float8e4)
            outb_sb = pp.tile([1, VPAD], dt.bfloat16)
            ones_sb = pp.tile([1, TILE_P], dt.bfloat16)
            nc.vector.memset(ones_sb[:], 1.0)
            nc.sync.dma_start(ident_sb[:], ident[:])
            nc.sync.dma_start(idx_sb[:], idx[:])
            nc.sync.dma_start(whh_sb[:], whh8[:])
            nc.sync.dma_start(bhn_sb[:], bhn[:])
            nc.sync.dma_start(h0_sb[:], h0[:])
            nc.sync.dma_start(h08_sb[:], h08[:])
            nc.sync.dma_start(outb_sb[:], outb[:])
            nc.sync.dma_start(wout_sb[:], wout8[:])

            xg_tiles = {}

            def gather(i):
                xrow = xg.tile([128, 3 * HID], dt.bfloat16, tag="xrow")
                nc.gpsimd.indirect_dma_start(
                    out=xrow[:], out_offset=None, in_=embgi[:],
                    in_offset=bass.IndirectOffsetOnAxis(
                        ap=idx_sb[:, i:i + 1], axis=0))
                xg_tiles[i] = xrow

            # state carried across schedule actions
            lbufs, sums_t, sums4_t, ar_t, c_t = {}, {}, {}, {}, {}

            def emit_rec(t):
                i = t // 4
                if t % 4 == 0:
                    for k in (1, 2):
                        if i + k < 32 and (i + k) not in xg_tiles:
                            gather(i + k)
                xrow = xg_tiles[i]
                ps = rps.tile([128, 2, 16, 16], dt.float32, tag="rec")
                tau, v2, nb = [], [], []
                for h in (0, 1):
                    sel = (t % 4) * 32 + 16 * h
                    cols = slice(t * 32 + 16 * h, t * 32 + 16 * h + 16)
                    pcols = slice((t - 1) * 32 + 16 * h,
                                  (t - 1) * 32 + 16 * h + 16)
                    # r,z gates: gi preload + W_hh accumulation (fp8 DR)
                    for mc in range(8):
                        nc.tensor.matmul(
                            ps[:, h, mc, :],
                            xrow[:, mc * 128:(mc + 1) * 128],
                            ident_sb[:, sel:sel + 16],
                            start=True, stop=False)
                        for j in (0, 1):
                            rhs = (h08_sb[:, 2 * j:2 * j + 2, 16 * h:16 * h + 16]
                                   if t == 0 else hsT8[:, 2 * j:2 * j + 2, pcols])
                            nc.tensor.matmul(
                                ps[:, h, mc, :],
                                whh_sb[:, 2 * j:2 * j + 2,
                                       mc * 128:(mc + 1) * 128],
                                rhs, perf_mode=DR,
                                start=False, stop=(j == 1))
                    # n gate: b_hn preload + W_hh accumulation
                    for c in range(4):
                        mc = 8 + c
                        nc.tensor.matmul(
                            ps[:, h, mc, :], ident_sb[:],
                            bhn_sb[:, c, :], start=True, stop=False)
                        for j in (0, 1):
                            rhs = (h08_sb[:, 2 * j:2 * j + 2, 16 * h:16 * h + 16]
                                   if t == 0 else hsT8[:, 2 * j:2 * j + 2, pcols])
                            nc.tensor.matmul(
                                ps[:, h, mc, :],
                                whh_sb[:, 2 * j:2 * j + 2,
                                       mc * 128:(mc + 1) * 128],
                                rhs, perf_mode=DR,
                                start=False, stop=(j == 1))
                    # 2*gi_n (doubled in the host table)
                    for c in range(4):
                        nc.tensor.matmul(
                            ps[:, h, 12 + c, :],
                            xrow[:, (8 + c) * 128:(9 + c) * 128],
                            ident_sb[:, sel:sel + 16],
                            start=True, stop=True)
                # tau = tanh(0.5*(gi+gh)) for r|z  (2*sigmoid(x)-1)
                for h in (0, 1):
                    tt = rp.tile([128, 8, 16], dt.float32, tag=f"tau{h}")
                    nc.scalar.activation(tt[:], ps[:, h, 0:8, :],
                                         AF.Tanh, scale=0.5)
                    tau.append(tt)
                for h in (0, 1):
                    # v2 = (1+tau_r) * (W_hh_n h + b_hn)   (= 2*r*(...))
                    vt = rp.tile([128, 4, 16], dt.float32, tag=f"v2{h}")
                    nc.vector.scalar_tensor_tensor(
                        vt[:], tau[h][:, 0:4, :], 1.0, ps[:, h, 8:12, :],
                        AL.add, AL.mult)
                    # t2 = v2 + 2*gi_n
                    t2 = rp.tile([128, 4, 16], dt.float32, tag=f"t2{h}")
                    nc.vector.tensor_tensor(t2[:], vt[:], ps[:, h, 12:16, :],
                                            AL.add)
                    v2.append(t2)
                for h in (0, 1):
                    nt_ = rp.tile([128, 4, 16], dt.float32, tag=f"n{h}")
                    nc.scalar.activation(nt_[:], v2[h][:], AF.Tanh, scale=0.5)
                    nb.append(nt_)
                for h in (0, 1):
                    cols = slice(t * 32 + 16 * h, t * 32 + 16 * h + 16)
                    pcols = slice((t - 1) * 32 + 16 * h,
                                  (t - 1) * 32 + 16 * h + 16)
                    hprev = (h0_sb[:, :, 16 * h:16 * h + 16] if t == 0
                             else hsT[:, :, pcols])
                    d = rp.tile([128, 4, 16], dt.float32, tag=f"d{h}")
                    nc.vector.tensor_tensor(d[:], hprev, nb[h][:], AL.subtract)
                    # zd2 = (1+tau_z)*d = 2*z*(h_prev - n)
                    zt = rp.tile([128, 4, 16], dt.float32, tag=f"zd{h}")
                    nc.vector.scalar_tensor_tensor(
                        zt[:], tau[h][:, 4:8, :], 1.0, d[:], AL.add, AL.mult)
                    # h_new = n + 0.5*zd2  -> hsT (bf16) and hsT8 (fp8)
                    nc.vector.scalar_tensor_tensor(
                        hsT[:, :, cols], zt[:], 0.5, nb[h][:], AL.mult, AL.add)
                    nc.vector.tensor_copy(hsT8[:, :, cols], hsT[:, :, cols])

            def emit_chunk(p, q):
                g, gloc = p // GT, p % GT
                if gloc == 0 and q == 0:
                    gsz = min(GT, NT - p)
                    lbufs[g] = lb.tile([128, GT, VPAD], dt.bfloat16,
                                       tag="lbuf", name="lbuf")
                    sums4_t[g] = {}
                if q == 0:
                    sums4_t[g][gloc] = sp.tile([128, 4], dt.float32,
                                               tag="sums4", name="sums4")
                ps = pps.tile([128, 2, 512], dt.float32, tag="pj")
                for s in (0, 1):
                    col0 = 1024 * q + 512 * s
                    nc.tensor.matmul(
                        ps[0:TILE_P, s, :], ones_sb[:],
                        outb_sb[:, col0:col0 + 512], start=True, stop=False)
                    for j in (0, 1):
                        nc.tensor.matmul(
                            ps[0:TILE_P, s, :],
                            hsT8[:, 2 * j:2 * j + 2, ts(p, TILE_P)],
                            wout_sb[:, 2 * j:2 * j + 2, col0:col0 + 512],
                            perf_mode=DR, start=False, stop=(j == 1))
                ldst = lbufs[g][0:TILE_P, gloc, 1024 * q:1024 * q + 1024]
                if (p + q) % 2 == 0:
                    nc.vector.tensor_copy(ldst, ps[0:TILE_P, :, :])
                else:
                    nc.scalar.copy(ldst, ps[0:TILE_P, :, :])
                esc = ep.tile([128, 1024], dt.bfloat16, tag="esc")
                nc.scalar.activation(
                    esc[0:TILE_P, :], ldst, AF.Exp,
                    accum_out=sums4_t[g][gloc][0:TILE_P, q:q + 1])
                if q == 3:
                    gsz = min(GT, NT - (p - gloc))
                    if gloc == 0:
                        sums_t[g] = sp.tile([128, GT], dt.float32,
                                            tag="sums", name="sums")
                    nc.vector.tensor_reduce(
                        sums_t[g][0:TILE_P, gloc:gloc + 1],
                        sums4_t[g][gloc][0:TILE_P, :],
                        mybir.AxisListType.X, AL.add)

            def emit_allreduce(g, gsz):
                arin = ad.tile([TILE_P, gsz], dt.float32, tag=f"arin{gsz}")
                arout = ad.tile([TILE_P, gsz], dt.float32, tag=f"arout{gsz}",
                                addr_space="Shared")
                nc.gpsimd.dma_start(arin[:], sums_t[g][0:TILE_P, 0:gsz])
                nc.gpsimd.collective_compute(
                    "AllReduce", AL.add,
                    replica_groups=[list(range(N_CORES))],
                    ins=[arin.opt()], outs=[arout.opt()])
                ar_t[g] = arout

            def emit_fetch_c(g, gsz):
                stot = sp.tile([128, GT], dt.float32, tag="stot")
                nc.sync.dma_start(stot[0:TILE_P, 0:gsz], ar_t[g][:])
                # negc = -ln(stot) = -(15*ln2 + ln1p(u)), u = stot/32768 - 1
                P = slice(0, TILE_P)
                u = sp.tile([128, GT], dt.float32, tag="u")
                nc.vector.tensor_scalar(u[P, 0:gsz], stot[P, 0:gsz],
                                        1.0 / 32768.0, -1.0, AL.mult, AL.add)
                # ln1p(u) = ((((0.2u-0.25 + 0)u + 1/3)u - 0.5)u + 1)u
                # via x_{k+1} = (x_k + c_k)*u steps (one fused stt each)
                acc = sp.tile([128, GT], dt.float32, tag="acc")
                nc.vector.tensor_scalar(acc[P, 0:gsz], u[P, 0:gsz],
                                        0.2, -0.25, AL.mult, AL.add)
                for k, cst in enumerate((0.0, 1.0 / 3.0, -0.5, 1.0)):
                    acc2 = sp.tile([128, GT], dt.float32, tag=f"acc{k % 2}b")
                    nc.vector.scalar_tensor_tensor(
                        acc2[P, 0:gsz], acc[P, 0:gsz], cst, u[P, 0:gsz],
                        AL.add, AL.mult)
                    acc = acc2
                negc = sp.tile([128, GT], dt.float32, tag="negc")
                nc.vector.tensor_scalar(negc[P, 0:gsz], acc[P, 0:gsz],
                                        -1.0, -15.0 * LN2, AL.mult, AL.add)
                c_t[g] = negc

            def emit_sub(g, p):
                gloc = p % GT
                o = op.tile([128, VPAD], dt.bfloat16, tag="o")
                if p % 2 == 0:
                    nc.vector.tensor_scalar(
                        o[0:TILE_P, :], lbufs[g][0:TILE_P, gloc, :],
                        c_t[g][0:TILE_P, gloc:gloc + 1], None, AL.add)
                else:
                    nc.scalar.activation(
                        o[0:TILE_P, :], lbufs[g][0:TILE_P, gloc, :],
                        AF.Identity, bias=c_t[g][0:TILE_P, gloc:gloc + 1])
                nc.sync.dma_start(out[ts(p, TILE_P), :], o[0:TILE_P, :])

            def run_sched(t):
                for action in sched.pop(t, []):
                    kind = action[0]
                    if kind == "chunk":
                        emit_chunk(action[1], action[2])
                    elif kind == "allreduce":
                        emit_allreduce(action[1], action[2])
                    elif kind == "fetch_c":
                        emit_fetch_c(action[1], action[2])
                    elif kind == "sub":
                        emit_sub(action[1], action[2])

            gather(0)
            for t in range(T):
                emit_rec(t)
                run_sched(t)
            for t in range(T, T + 40):
                run_sched(t)
            assert not sched, f"unemitted schedule entries: {sorted(sched)}"
    nc.compile()
    return nc


def _chunkT(w):  # [512, M] -> [128, KC, M]
    return np.ascontiguousarray(w.reshape(KC, 128, -1).transpose(1, 0, 2))


def prep_inputs(target, encoder_hidden, emb_weight, W_ih, W_hh, b_ih, b_hh,
                out_W, out_b):
    tok = np.ascontiguousarray(target[:, :T].T).reshape(-1).astype(np.int32)
    tok_pad = np.zeros(4096, np.int32)
    tok_pad[:POS] = tok
    idx = np.ascontiguousarray(tok_pad.reshape(32, 128).T)
    ident = np.eye(128, dtype=BF16)

    # embgi[v] = emb[v] @ W_ih.T + b_ih (+ b_hh for r,z); n part doubled.
    g = emb_weight.astype(F32) @ W_ih.astype(F32).T
    g[:, :2 * HID] += (b_ih[:2 * HID] + b_hh[:2 * HID]).astype(F32)
    g[:, 2 * HID:] += b_ih[2 * HID:].astype(F32)
    g[:, 2 * HID:] *= 2.0
    embgi = g.astype(BF16)

    whhT = _chunkT(np.ascontiguousarray(W_hh.T).astype(F32))
    whh8 = whhT.astype(FP8)
    bhn = np.ascontiguousarray(np.broadcast_to(
        b_hh[2 * HID:].astype(F32).reshape(KC, 128).T[:, :, None],
        (128, KC, 16))).astype(BF16)
    h0f = _chunkT(np.ascontiguousarray(encoder_hidden[0].T).astype(F32))
    h0 = h0f.astype(BF16)
    h08 = h0f.astype(FP8)

    outWT = np.ascontiguousarray(out_W.T.astype(F32))  # [512, 32000]

    in_maps = []
    for j in range(N_CORES):
        sl = slice(j * VSHARD, (j + 1) * VSHARD)
        wpad = np.zeros((HID, VPAD), F32)
        wpad[:, :VSHARD] = outWT[:, sl]
        bpad = np.full((1, VPAD), -30.0, F32)
        bpad[0, :VSHARD] = out_b[sl]
        in_maps.append({
            "idx": idx, "ident": ident, "embgi": embgi, "whh8": whh8,
            "bhn": bhn, "h0": h0, "h08": h08,
            "wout8": _chunkT(wpad).astype(FP8),
            "outb": bpad.astype(BF16),
        })
    return in_maps


_NC_CACHE = {}


def kernel(**inputs):
    inputs = {k: np.asarray(v) for k, v in inputs.items()}
    target = inputs["target"].astype(np.int32)
    assert target.shape[1] - 1 == T
    if "nc" not in _NC_CACHE:
        _NC_CACHE["nc"] = build_nc()
    nc = _NC_CACHE["nc"]
    in_maps = prep_inputs(
        target, inputs["encoder_hidden"].astype(F32),
        inputs["emb_weight"].astype(F32), inputs["W_ih"].astype(F32),
        inputs["W_hh"].astype(F32), inputs["b_ih"].astype(F32),
        inputs["b_hh"].astype(F32), inputs["out_W"].astype(F32),
        inputs["out_b"].astype(F32))
    res = run_bass_kernel_spmd(nc, in_maps, list(range(N_CORES)))
    full = np.concatenate(
        [res.results[j]["out"][:, :VSHARD].astype(F32)
         for j in range(N_CORES)], axis=1)
    return np.ascontiguousarray(full.reshape(T, BATCH, VOCAB))
